# revision 5
# baseline (speedup 1.0000x reference)
"""Pixel-RNN control kernel for TRN2 (8 NeuronCores, batch-sharded).

Math: h_t = tanh(x_t @ W_ih^T + h_{t-1} @ W_hh^T + b), out_t = sigmoid(10*h_t).
With W_hh = I, b = 0, W_ih = diag(s), pre-scale x by s on the host (exact):
    h_t = tanh(x'_t + h_{t-1})      (pure elementwise add chain)
    out_t = sigmoid(10 * h_t)       (ACT sigmoid, immediate scale)
    hidden = h_T

Layout per core: component 0 lives in partitions 0-63, component 1 in
64-127; partition p = comp*64 + q holds pixels [q*512, (q+1)*512) of its
component, so every step is one contiguous FD=512 instruction per engine.
ACT alternates tanh/sigmoid; DVE add runs in the sigmoid shadow. Output
DMAs are issued at 2-step granularity to avoid head-of-line blocking of
the next block's input DMAs on the single hardware queue.
"""

import numpy as np

T = 128          # sequence length
B = 262144       # total pixels
NCORES = 8
BC = B // NCORES  # 32768 pixels per core
P = 128           # SBUF partitions
Q = 64            # pixel-groups per component (P = 2 * Q)
F = BC // Q       # 512 floats per partition per time step
TB = 8            # time-block (steps per x/y tile)
HB = TB // 2      # half-block (steps per steady-state x DMA)
NBLK = T // TB

_NC_CACHE = {}


def _build_nc():
    import concourse.bass as bass  # noqa: F401
    import concourse.tile as tile
    from concourse import bacc, mybir

    f32 = mybir.dt.float32
    AF = mybir.ActivationFunctionType

    nc = bacc.Bacc(None, target_bir_lowering=False, debug=False)
    x = nc.dram_tensor("x", [P, T * F], f32, kind="ExternalInput")
    h0 = nc.dram_tensor("h0", [P, F], f32, kind="ExternalInput")
    out = nc.dram_tensor("out", [P, T * F], f32, kind="ExternalOutput")
    hid = nc.dram_tensor("hidden", [P, F], f32, kind="ExternalOutput")

    HF = HB * F   # half-block floats
    QF = 2 * F    # 2-step piece

    with tile.TileContext(nc) as tc:
        with (
            tc.tile_pool(name="xp", bufs=3) as xp,
            tc.tile_pool(name="gp", bufs=3) as gp,
            tc.tile_pool(name="yp", bufs=3) as yp,
            tc.tile_pool(name="up", bufs=3) as up,
            tc.tile_pool(name="smp", bufs=1) as smp,
        ):
            ht = smp.tile([P, F], f32, tag="ht")
            nc.sync.dma_start(ht[:, :1], h0[:, :1])
            # warm both ACT function tables while input DMAs stream
            warm = smp.tile([P, 1], f32, tag="warm")
            nc.scalar.activation(warm[:], ht[:, :1], AF.Tanh)
            nc.scalar.activation(warm[:], ht[:, :1], AF.Sigmoid)
            nc.sync.dma_start(ht[:, 1:], h0[:, 1:])

            # block-0 input split fine so compute starts early
            xtiles = [None] * NBLK
            xb0 = xp.tile([P, TB * F], f32)
            for q in range(4):
                nc.sync.dma_start(
                    xb0[:, q * QF:(q + 1) * QF], x[:, q * QF:(q + 1) * QF]
                )
            xtiles[0] = xb0

            gprev = ht[:]
            for b in range(NBLK):
                t0 = b * TB
                if b + 1 < NBLK:
                    xn = xp.tile([P, TB * F], f32)
                    base = (b + 1) * TB * F
                    nc.sync.dma_start(xn[:, :HF], x[:, base:base + HF])
                    nc.sync.dma_start(
                        xn[:, HF:], x[:, base + HF:base + TB * F]
                    )
                    xtiles[b + 1] = xn
                xb = xtiles[b]
                xtiles[b] = None
                yb = yp.tile([P, TB * F], f32)
                for j in range(TB):
                    u = up.tile([P, F], f32)
                    nc.vector.tensor_add(u[:], xb[:, j * F:(j + 1) * F], gprev)
                    g = gp.tile([P, F], f32)
                    nc.scalar.activation(g[:], u[:], AF.Tanh)
                    nc.scalar.activation(
                        yb[:, j * F:(j + 1) * F], g[:], AF.Sigmoid,
                        scale=10.0,
                    )
                    gprev = g[:]
                    if j % 2 == 1:
                        lo = (j - 1) * F
                        nc.sync.dma_start(
                            out[:, t0 * F + lo:t0 * F + lo + QF],
                            yb[:, lo:lo + QF],
                        )

            nc.sync.dma_start(hid[:], gprev)

    nc.compile()
    return nc


def _get_nc():
    if "nc" not in _NC_CACHE:
        _NC_CACHE["nc"] = _build_nc()
    return _NC_CACHE["nc"]


def _make_in_maps(x, h, s0, s1):
    # partition p = comp*64 + q holds pixels [q*512, (q+1)*512) of its comp
    xt = np.ascontiguousarray(
        x.reshape(T, NCORES, Q, F, 2).transpose(1, 4, 2, 0, 3)
    ).reshape(NCORES, 2, Q, T * F)
    if s0 != 1.0:
        xt[:, 0] *= np.float32(s0)
    if s1 != 1.0:
        xt[:, 1] *= np.float32(s1)
    xt = xt.reshape(NCORES, P, T * F)
    hp = np.ascontiguousarray(
        h[0].reshape(NCORES, Q, F, 2).transpose(0, 3, 1, 2)
    ).reshape(NCORES, P, F)
    return [{"x": xt[c], "h0": hp[c]} for c in range(NCORES)]


def _run(nc, in_maps, trace=False):
    from concourse.bass_utils import run_bass_kernel_spmd
    return run_bass_kernel_spmd(nc, in_maps, list(range(NCORES)), trace=trace)


def _assemble(results):
    out = np.concatenate(
        [
            results[c]["out"].reshape(2, Q, T, F).transpose(2, 1, 3, 0)
            .reshape(T, BC, 2)
            for c in range(NCORES)
        ],
        axis=1,
    )
    hidden = np.concatenate(
        [
            results[c]["hidden"].reshape(2, Q, F).transpose(1, 2, 0)
            .reshape(BC, 2)
            for c in range(NCORES)
        ],
        axis=0,
    )[None]
    return np.ascontiguousarray(out), np.ascontiguousarray(hidden)


def _numpy_ref(x, h, W_ih, W_hh, b_ih, b_hh):
    hp = h[0].astype(np.float32)
    bias = (b_ih + b_hh).astype(np.float32)
    out = np.empty_like(x)
    for t in range(x.shape[0]):
        hp = np.tanh(x[t] @ W_ih.T + hp @ W_hh.T + bias)
        out[t] = 1.0 / (1.0 + np.exp(-10.0 * hp))
    return out, hp[None]


def kernel(x, h, W_ih, W_hh, b_ih, b_hh):
    x = np.ascontiguousarray(np.asarray(x, dtype=np.float32))
    h = np.ascontiguousarray(np.asarray(h, dtype=np.float32))
    W_ih = np.asarray(W_ih, dtype=np.float32)
    W_hh = np.asarray(W_hh, dtype=np.float32)
    b_ih = np.asarray(b_ih, dtype=np.float32)
    b_hh = np.asarray(b_hh, dtype=np.float32)

    fast = (
        x.shape == (T, B, 2)
        and h.shape == (1, B, 2)
        and np.array_equal(W_hh, np.eye(2, dtype=np.float32))
        and np.all(b_ih + b_hh == 0.0)
        and W_ih[0, 1] == 0.0 and W_ih[1, 0] == 0.0
    )
    if not fast:
        return _numpy_ref(x, h, W_ih, W_hh, b_ih, b_hh)

    s0 = float(W_ih[0, 0])
    s1 = float(W_ih[1, 1])
    nc = _get_nc()
    in_maps = _make_in_maps(x, h, s0, s1)
    results = _run(nc, in_maps).results
    return _assemble(results)


# revision 7
# speedup vs baseline: 1.0357x; 1.0357x over previous
"""Pixel-RNN control kernel for TRN2 (8 NeuronCores, batch-sharded).

Math: h_t = tanh(x_t @ W_ih^T + h_{t-1} @ W_hh^T + b), out_t = sigmoid(10*h_t).
With W_hh = I, b = 0, W_ih = diag(s), pre-scale x by s on the host (exact):
    h_t = tanh(x'_t + h_{t-1})      (pure elementwise add chain)
    out_t = sigmoid(10 * h_t)       (ACT sigmoid, immediate scale)
    hidden = h_T

Layout per core: component 0 lives in partitions 0-63, component 1 in
64-127; partition p = comp*64 + q holds pixels [q*512, (q+1)*512) of its
component, so every step is one contiguous FD=512 instruction per engine.
ACT alternates tanh/sigmoid; DVE add runs in the sigmoid shadow. Input
DMAs stream on the SP hardware queue, output DMAs drain on the GpSimd
software-DGE queue, so the two streams never block each other.
"""

import numpy as np

T = 128          # sequence length
B = 262144       # total pixels
NCORES = 8
BC = B // NCORES  # 32768 pixels per core
P = 128           # SBUF partitions
Q = 64            # pixel-groups per component (P = 2 * Q)
F = BC // Q       # 512 floats per partition per time step
TB = 8            # time-block (steps per x/y tile)
HB = TB // 2      # half-block (steps per steady-state x DMA)
NBLK = T // TB

_NC_CACHE = {}


def _build_nc():
    import concourse.bass as bass  # noqa: F401
    import concourse.tile as tile
    from concourse import bacc, mybir

    f32 = mybir.dt.float32
    AF = mybir.ActivationFunctionType

    nc = bacc.Bacc(None, target_bir_lowering=False, debug=False)
    x = nc.dram_tensor("x", [P, T * F], f32, kind="ExternalInput")
    h0 = nc.dram_tensor("h0", [P, F], f32, kind="ExternalInput")
    out = nc.dram_tensor("out", [P, T * F], f32, kind="ExternalOutput")
    hid = nc.dram_tensor("hidden", [P, F], f32, kind="ExternalOutput")

    HF = HB * F   # half-block floats
    QF = 2 * F    # 2-step piece

    with tile.TileContext(nc) as tc:
        with (
            tc.tile_pool(name="xp", bufs=3) as xp,
            tc.tile_pool(name="gp", bufs=3) as gp,
            tc.tile_pool(name="yp", bufs=4) as yp,
            tc.tile_pool(name="up", bufs=2, space="PSUM") as up,
            tc.tile_pool(name="smp", bufs=1) as smp,
        ):
            ht = smp.tile([P, F], f32, tag="ht")
            nc.sync.dma_start(ht[:, :1], h0[:, :1])
            # warm both ACT function tables while input DMAs stream
            warm = smp.tile([P, 1], f32, tag="warm")
            nc.scalar.activation(warm[:], ht[:, :1], AF.Tanh)
            nc.scalar.activation(warm[:], ht[:, :1], AF.Sigmoid)
            nc.sync.dma_start(ht[:, 1:], h0[:, 1:])

            # block-0 input split fine so compute starts early
            xtiles = [None] * NBLK
            xb0 = xp.tile([P, TB * F], f32)
            for q in range(4):
                nc.sync.dma_start(
                    xb0[:, q * QF:(q + 1) * QF], x[:, q * QF:(q + 1) * QF]
                )
            xtiles[0] = xb0

            gprev = ht[:]
            for b in range(NBLK):
                t0 = b * TB
                last = b == NBLK - 1
                if not last:
                    xn = xp.tile([P, TB * F], f32)
                    base = (b + 1) * TB * F
                    nc.sync.dma_start(xn[:], x[:, base:base + TB * F])
                    xtiles[b + 1] = xn
                xb = xtiles[b]
                xtiles[b] = None
                yb = yp.tile([P, TB * F], f32)
                for j in range(TB):
                    u = up.tile([P, F], f32)
                    nc.vector.tensor_add(u[:], xb[:, j * F:(j + 1) * F], gprev)
                    g = gp.tile([P, F], f32)
                    nc.scalar.activation(g[:], u[:], AF.Tanh)
                    nc.scalar.activation(
                        yb[:, j * F:(j + 1) * F], g[:], AF.Sigmoid,
                        scale=10.0,
                    )
                    gprev = g[:]
                    # drain outputs on the GpSimd SWDGE queue
                    if j == HB - 1:
                        nc.gpsimd.dma_start(
                            out[:, t0 * F:t0 * F + HF], yb[:, :HF]
                        )
                    elif j == TB - 1:
                        if last:
                            pass  # handled below in fine pieces
                        else:
                            nc.gpsimd.dma_start(
                                out[:, t0 * F + HF:(t0 + TB) * F], yb[:, HF:]
                            )
                    elif last and j in (5, 6):
                        lo = (4 if j == 5 else 6) * F
                        hi = (j + 1) * F
                        nc.gpsimd.dma_start(
                            out[:, t0 * F + lo:t0 * F + hi], yb[:, lo:hi]
                        )
                if last:
                    nc.gpsimd.dma_start(
                        out[:, (t0 + TB - 1) * F:(t0 + TB) * F],
                        yb[:, (TB - 1) * F:],
                    )

            nc.sync.dma_start(hid[:], gprev)

    nc.compile()
    return nc


def _get_nc():
    if "nc" not in _NC_CACHE:
        _NC_CACHE["nc"] = _build_nc()
    return _NC_CACHE["nc"]


def _make_in_maps(x, h, s0, s1):
    # partition p = comp*64 + q holds pixels [q*512, (q+1)*512) of its comp
    xt = np.ascontiguousarray(
        x.reshape(T, NCORES, Q, F, 2).transpose(1, 4, 2, 0, 3)
    ).reshape(NCORES, 2, Q, T * F)
    if s0 != 1.0:
        xt[:, 0] *= np.float32(s0)
    if s1 != 1.0:
        xt[:, 1] *= np.float32(s1)
    xt = xt.reshape(NCORES, P, T * F)
    hp = np.ascontiguousarray(
        h[0].reshape(NCORES, Q, F, 2).transpose(0, 3, 1, 2)
    ).reshape(NCORES, P, F)
    return [{"x": xt[c], "h0": hp[c]} for c in range(NCORES)]


def _run(nc, in_maps, trace=False):
    from concourse.bass_utils import run_bass_kernel_spmd
    return run_bass_kernel_spmd(nc, in_maps, list(range(NCORES)), trace=trace)


def _assemble(results):
    out = np.concatenate(
        [
            results[c]["out"].reshape(2, Q, T, F).transpose(2, 1, 3, 0)
            .reshape(T, BC, 2)
            for c in range(NCORES)
        ],
        axis=1,
    )
    hidden = np.concatenate(
        [
            results[c]["hidden"].reshape(2, Q, F).transpose(1, 2, 0)
            .reshape(BC, 2)
            for c in range(NCORES)
        ],
        axis=0,
    )[None]
    return np.ascontiguousarray(out), np.ascontiguousarray(hidden)


def _numpy_ref(x, h, W_ih, W_hh, b_ih, b_hh):
    hp = h[0].astype(np.float32)
    bias = (b_ih + b_hh).astype(np.float32)
    out = np.empty_like(x)
    for t in range(x.shape[0]):
        hp = np.tanh(x[t] @ W_ih.T + hp @ W_hh.T + bias)
        out[t] = 1.0 / (1.0 + np.exp(-10.0 * hp))
    return out, hp[None]


def kernel(x, h, W_ih, W_hh, b_ih, b_hh):
    x = np.ascontiguousarray(np.asarray(x, dtype=np.float32))
    h = np.ascontiguousarray(np.asarray(h, dtype=np.float32))
    W_ih = np.asarray(W_ih, dtype=np.float32)
    W_hh = np.asarray(W_hh, dtype=np.float32)
    b_ih = np.asarray(b_ih, dtype=np.float32)
    b_hh = np.asarray(b_hh, dtype=np.float32)

    fast = (
        x.shape == (T, B, 2)
        and h.shape == (1, B, 2)
        and np.array_equal(W_hh, np.eye(2, dtype=np.float32))
        and np.all(b_ih + b_hh == 0.0)
        and W_ih[0, 1] == 0.0 and W_ih[1, 0] == 0.0
    )
    if not fast:
        return _numpy_ref(x, h, W_ih, W_hh, b_ih, b_hh)

    s0 = float(W_ih[0, 0])
    s1 = float(W_ih[1, 1])
    nc = _get_nc()
    in_maps = _make_in_maps(x, h, s0, s1)
    results = _run(nc, in_maps).results
    return _assemble(results)
